# revision 18
# baseline (speedup 1.0000x reference)
"""Kandinsky5Attention Bass/Tile kernel for 8 Trainium2 NeuronCores — v5.

Sharding: core = (batch b, head-group g): 2 batches x 4 groups of 4 heads.
Each core computes q/k/v for its 512 features of its batch, attention for
its 4 heads, and a partial output projection over its 512 contraction dims.
Host sums the 4 partials per batch and adds the output bias.

Key points (all empirically error-budgeted — fp8 variants exceed the 2e-2
gate because this problem's random rotary tensor amplifies scores to ~16,
so softmax is sharply peaked and per-term quantization error does not
average out; all-bf16 lands at ~0.9%):
- every matmul runs on bf16 operands (full PE rate, same as f32r).
- everything stays in SBUF between phases (no DRAM q/k/v spills);
  x^T is streamed per head-pass in 512-column chunks.
- softmax exp is batched 1024-wide out of 2-bank PSUM score tiles, and
  es lives in a small rolling window (z/attn@V consume each exp group
  one group behind the scores so the PE never waits on the scalar engine).
- RMSNorm: sum-of-squares via GpSimd partition_all_reduce,
  rsqrt = exp(-0.5*ln(var+eps)) on ScalarE so the only activation table
  set ever needed is natural_log_exp (no table thrashing);
  q_norm_w/k_norm_w are folded into the RoPE tables on the host.
- elementwise work is spread across DVE and GpSimd.
- emission interleaves QK-chunks, attention tiles and the output
  projection so all engines pipeline; one 8-bank PSUM plan covers all
  phases (tags shared across phases with disjoint lifetimes).

Weight columns are permuted per head (even dims then odd dims) so RoPE
operates on contiguous partition blocks [0:64] / [64:128].
"""
import math
import os

import numpy as np
import ml_dtypes

import concourse.bass as bass
import concourse.bass_isa as bass_isa
import concourse.mybir as mybir
import concourse.tile as tile
from concourse import bacc
from concourse.bass_utils import run_bass_kernel_spmd

B, S, C, HD = 2, 2048, 2048, 128
H = C // HD            # 16 heads
HG = 4                 # head groups (cores per batch)
HPG = H // HG          # 4 heads per group
GF = HPG * HD          # 512 features per group
EPS = float(np.finfo(np.float32).eps)
N_CORES = 8
NCC = C // 128         # 16 chunks over the C contraction
SQ_T = 512             # q tile
N_SQ = S // SQ_T       # 4
NSK2 = S // 256        # 8 exp groups (2 k-chunks each) per q tile

F32 = mybir.dt.float32
F32R = mybir.dt.float32r
BF16 = mybir.dt.bfloat16
AF = mybir.ActivationFunctionType
ALU = mybir.AluOpType
ISCALE = 1.0 / math.sqrt(HD)
EBIAS = -7.5           # exp(s*ISCALE + EBIAS): bounded well inside bf16

BF16NP = ml_dtypes.bfloat16


def _prefer_lnexp_table():
    """Make natural_log_exp_and_others the only table set advertising
    Exp/Ln (canonical order preserved so set ids still match
    act_info.json); every activation we use then lives in one set and the
    compiler emits a single ACT_TABLE_LOAD instead of thrashing."""
    import concourse.hw_specs as hws
    import concourse.bacc as bacc_mod

    orig = hws.get_activation_tables

    def patched(arch):
        t = orig(arch)
        pref = "natural_log_exp_and_others"
        if pref not in t:
            return t
        exp = mybir.ActivationFunctionType.Exp
        ln = mybir.ActivationFunctionType.Ln
        return {
            k: (set(v) if k == pref else set(v) - {exp, ln})
            for k, v in t.items()
        }

    return hws, bacc_mod, orig, patched


def build_program():
    hws, bacc_mod, _orig_tables, _patched = _prefer_lnexp_table()
    hws.get_activation_tables = _patched
    bacc_mod.get_activation_tables = _patched
    try:
        return _build_program_inner()
    finally:
        hws.get_activation_tables = _orig_tables
        bacc_mod.get_activation_tables = _orig_tables


def _build_program_inner():
    nc = bacc.Bacc("TRN2", target_bir_lowering=False, debug=False,
                   num_devices=N_CORES)

    xbd = nc.dram_tensor("xb", [128, NCC, S], BF16, kind="ExternalInput")
    wqd = nc.dram_tensor("wqb", [128, NCC, GF], BF16, kind="ExternalInput")
    wkd = nc.dram_tensor("wkb", [128, NCC, GF], BF16, kind="ExternalInput")
    wvd = nc.dram_tensor("wvb", [128, NCC, GF], BF16, kind="ExternalInput")
    wobd = nc.dram_tensor("wob", [128, HPG, C], BF16, kind="ExternalInput")
    ropeqd = nc.dram_tensor("ropeq", [2, 2, HD // 2, S], BF16,
                            kind="ExternalInput")
    ropekd = nc.dram_tensor("ropek", [2, 2, HD // 2, S], BF16,
                            kind="ExternalInput")
    bqd = nc.dram_tensor("bq", [HPG, HD], F32, kind="ExternalInput")
    bkd = nc.dram_tensor("bk", [HPG, HD], F32, kind="ExternalInput")
    onbfd = nc.dram_tensor("onbf", [1, 128], BF16, kind="ExternalInput")
    cstd = nc.dram_tensor("csts", [3, 128], F32, kind="ExternalInput")
    # csts row 0: 1.0 (rzb stationary); row 1: eps; row 2: EBIAS
    outd = nc.dram_tensor("out", [S, C], BF16, kind="ExternalOutput")
    dbg = os.environ.get("K5_DEBUG") == "1"
    if dbg:
        dbg_qh = nc.dram_tensor("dbg_qh", [128, 2, S], BF16,
                                kind="ExternalOutput")
        dbg_kh = nc.dram_tensor("dbg_kh", [128, 2, S], BF16,
                                kind="ExternalOutput")
        dbg_v = nc.dram_tensor("dbg_v", [128, NCC, GF], BF16,
                               kind="ExternalOutput")
        dbg_oT = nc.dram_tensor("dbg_oT", [128, HPG, S], BF16,
                                kind="ExternalOutput")

    with tile.TileContext(nc) as tc, \
            nc.allow_low_precision(reason="bf16 compute within tolerance"):
        with tc.tile_pool(name="glob", bufs=1) as glob:
            ones_col_bf = glob.tile([128, 1], BF16)
            nc.sync.dma_start(out=ones_col_bf,
                              in_=onbfd[0:1, :].rearrange("o d -> d o"))
            one_row_f = glob.tile([1, 128], F32R)
            nc.sync.dma_start(out=one_row_f, in_=cstd[0:1, :].bitcast(F32R))
            eps_t = glob.tile([128, 1], F32)
            nc.sync.dma_start(out=eps_t,
                              in_=cstd[1:2, :].rearrange("o d -> d o"))
            ebias_t = glob.tile([128, 1], F32)
            nc.sync.dma_start(out=ebias_t,
                              in_=cstd[2:3, :].rearrange("o d -> d o"))
            bq_t = glob.tile([128, HPG], F32)
            nc.sync.dma_start(out=bq_t, in_=bqd[:, :].rearrange("h d -> d h"))
            bk_t = glob.tile([128, HPG], F32)
            nc.sync.dma_start(out=bk_t, in_=bkd[:, :].rearrange("h d -> d h"))
            Rq = {}
            Rk = {}
            rope_loads = []
            for nm, dram, store in (("q", ropeqd, Rq), ("k", ropekd, Rk)):
                for r in range(2):
                    rt = glob.tile([128, S], BF16, tag=f"rope_{nm}{r}",
                                   name=f"rope_{nm}{r}")
                    rope_loads.append((rt, dram, r))
                    store[r] = rt

            # persistent intermediates
            v_t = glob.tile([128, NCC, GF], BF16)    # v[s, d]  (s-chunked)
            qh_t = glob.tile([128, 2, S], BF16)      # q^T[d, s], head slot h%2
            kh_t = glob.tile([128, 2, S], BF16)
            oTb_t = glob.tile([128, HPG, S], BF16)   # o^T / z
            wq_s = glob.tile([128, NCC, GF], BF16)
            wk_s = glob.tile([128, NCC, GF], BF16)
            wv_s = glob.tile([128, NCC, GF], BF16)
            wob_s = glob.tile([128, HPG, C], BF16)

            with (
                tc.tile_pool(name="pX", bufs=1) as pX,
                tc.tile_pool(name="pBw", bufs=2) as pBw,
                tc.tile_pool(name="pC", bufs=1) as pC,
                tc.tile_pool(name="pD", bufs=3) as pD,
                tc.tile_pool(name="ps", bufs=1, space="PSUM") as ps,
            ):
                def load_x(t):
                    # stream x^T columns [t*512, (t+1)*512) for one pass,
                    # split so the first contraction chunks land early
                    xt = pX.tile([128, NCC, SQ_T], BF16, tag="xt", bufs=2)
                    tsl = slice(t * SQ_T, (t + 1) * SQ_T)
                    for c0 in range(0, NCC, 4):
                        nc.sync.dma_start(out=xt[:, c0:c0 + 4],
                                          in_=xbd[:, c0:c0 + 4, tsl])
                    return xt

                # first pass: interleave x chunks with the wq chunks the
                # first accumulation chain consumes, so PE starts early
                xt0 = pX.tile([128, NCC, SQ_T], BF16, tag="xt", bufs=2,
                              name="xt0")
                for c0 in range(0, NCC, 4):
                    for cc in range(c0, c0 + 4):
                        nc.sync.dma_start(out=wq_s[:, cc], in_=wqd[:, cc])
                    nc.sync.dma_start(out=xt0[:, c0:c0 + 4],
                                      in_=xbd[:, c0:c0 + 4, 0:SQ_T])
                for cc in range(NCC):
                    nc.sync.dma_start(out=wk_s[:, cc], in_=wkd[:, cc])
                for rt, dram, r in rope_loads:
                    nc.sync.dma_start(out=rt[0:64, :], in_=dram[r, 0])
                    nc.sync.dma_start(out=rt[64:128, :], in_=dram[r, 1])
                for cc in range(NCC):
                    nc.sync.dma_start(out=wv_s[:, cc], in_=wvd[:, cc])
                nc.sync.dma_start(out=wob_s, in_=wobd[:, :, :])

                def emit_A(xt, t, i):
                    # V projection for s-rows [t*512 + 128*i, +128)
                    ssl = slice(i * 128, (i + 1) * 128)
                    vp = ps.tile([128, GF], F32, tag=("orz" if i % 2 else "zvp"),
                                 bufs=1)
                    for cc in range(NCC):
                        nc.tensor.matmul(vp[:], xt[:, cc, ssl], wv_s[:, cc, :],
                                         start=(cc == 0),
                                         stop=(cc == NCC - 1))
                    nc.vector.tensor_copy(v_t[:, t * 4 + i, :], vp[:])

                def emit_B(h, t, xt):
                    # Q and K chunk [128, 512] for head h, s-cols t
                    hsl = slice(h * HD, (h + 1) * HD)
                    tsl = slice(t * SQ_T, (t + 1) * SQ_T)
                    hs = h % 2
                    for w_s, Rx, b_t, dsth in (
                        (wq_s, Rq, bq_t, qh_t),
                        (wk_s, Rk, bk_t, kh_t),
                    ):
                        qk = ps.tile([128, SQ_T], F32, tag="qk", bufs=2)
                        for cc in range(NCC):
                            nc.tensor.matmul(qk[:], w_s[:, cc, hsl],
                                             xt[:, cc, :],
                                             start=(cc == 0),
                                             stop=(cc == NCC - 1))
                        raw = pBw.tile([128, SQ_T], F32, tag="raw")
                        nc.vector.tensor_scalar_add(raw[:], qk[:],
                                                    b_t[:, h:h + 1])
                        sq2 = pBw.tile([128, SQ_T], F32, tag="sq2")
                        nc.gpsimd.tensor_mul(sq2[:], raw[:], raw[:])
                        ssqb = pBw.tile([128, SQ_T], F32, tag="ssqb")
                        nc.gpsimd.partition_all_reduce(
                            ssqb[:], sq2[:], 128, bass_isa.ReduceOp.add)
                        lv = pBw.tile([128, SQ_T], F32, tag="sq2", name="lv")
                        nc.scalar.activation(lv[:], ssqb[:], AF.Ln,
                                             scale=1.0 / HD, bias=eps_t[:])
                        rs = pBw.tile([128, SQ_T], F32, tag="rs")
                        nc.scalar.activation(rs[:], lv[:], AF.Exp, scale=-0.5)
                        qn = pBw.tile([128, SQ_T], F32, tag="qn")
                        nc.vector.tensor_mul(qn[:], raw[:], rs[:])
                        ta = pBw.tile([128, SQ_T], F32, tag="ta")
                        tb = pBw.tile([128, SQ_T], F32, tag="tb")
                        nc.vector.tensor_mul(ta[:], Rx[0][:, tsl], qn[:])
                        nc.gpsimd.tensor_mul(tb[:], Rx[1][:, tsl], qn[:])
                        m1 = pBw.tile([128, SQ_T], F32, tag="m1")
                        nc.sync.dma_start(out=m1[0:64, :], in_=ta[64:128, :])
                        nc.sync.dma_start(out=m1[64:128, :], in_=tb[0:64, :])
                        nc.vector.tensor_add(dsth[0:64, hs, tsl],
                                             ta[0:64, :], m1[0:64, :])
                        nc.vector.tensor_add(dsth[64:128, hs, tsl],
                                             tb[64:128, :], m1[64:128, :])

                def emit_C(h, sq):
                    qsl = slice(sq * SQ_T, (sq + 1) * SQ_T)
                    hsl = slice(h * HD, (h + 1) * HD)
                    hs = h % 2
                    z_full = ps.tile([128, SQ_T], F32, tag="zvp", bufs=1,
                                     name="z_full")
                    z_ps = z_full[0:1, :]
                    o_ps = ps.tile([128, SQ_T], F32, tag="orz", bufs=1)

                    def zo(esj, sk):
                        # consume exp chunk sk: z += 1^T es, o += V^T es
                        st = (sk == 0)
                        sp = (sk == 2 * NSK2 - 1)
                        nc.tensor.matmul(z_ps[:], ones_col_bf[:],
                                         esj, start=st, stop=sp)
                        nc.tensor.matmul(o_ps[:], v_t[:, sk, hsl],
                                         esj, start=st, stop=sp)

                    pend = []
                    for g in range(NSK2):
                        sc_ps = ps.tile([128, 2, SQ_T], F32, tag="scs",
                                        bufs=2)
                        for j in range(2):
                            sk = g * 2 + j
                            nc.tensor.matmul(
                                sc_ps[:, j, :],
                                kh_t[:, hs, sk * 128:(sk + 1) * 128],
                                qh_t[:, hs, qsl])
                        es = pC.tile([128, 2, SQ_T], BF16, tag="es", bufs=4)
                        nc.scalar.activation(es[:], sc_ps[:], AF.Exp,
                                             scale=ISCALE, bias=ebias_t[:])
                        pend.append((es, g))
                        if len(pend) > 2:
                            e0, g0 = pend.pop(0)
                            for j in range(2):
                                zo(e0[:, j, :], g0 * 2 + j)
                    for e0, g0 in pend:
                        for j in range(2):
                            zo(e0[:, j, :], g0 * 2 + j)
                    rz = pC.tile([1, SQ_T], F32R, tag="rz", bufs=1)
                    nc.vector.reciprocal(rz[:], z_ps[:])
                    oe = pC.tile([128, SQ_T], F32, tag="oe", bufs=1)
                    nc.vector.tensor_copy(oe[:], o_ps[:])
                    rzb = ps.tile([128, SQ_T], F32, tag="orz", bufs=1)
                    nc.tensor.matmul(rzb[:], one_row_f[:], rz[:])
                    nc.vector.tensor_mul(oTb_t[:, h, qsl], oe[:], rzb[:])

                def emit_D(sq):
                    for st in range(sq * 4, sq * 4 + 4):
                        stsl = slice(st * 128, (st + 1) * 128)
                        for jc in range(C // SQ_T):
                            jsl = slice(jc * SQ_T, (jc + 1) * SQ_T)
                            op = ps.tile([128, SQ_T], F32, tag="qk", bufs=2)
                            for hh in range(HPG):
                                nc.tensor.matmul(op[:],
                                                 oTb_t[:, hh, stsl],
                                                 wob_s[:, hh, jsl],
                                                 start=(hh == 0),
                                                 stop=(hh == HPG - 1))
                            oe3 = pD.tile([128, SQ_T], BF16, tag="oe3")
                            nc.vector.tensor_copy(oe3[:], op[:])
                            nc.sync.dma_start(out=outd[stsl, jsl], in_=oe3[:])

                # ---- schedule ----
                # pass 0: B(head 0) + A;  pass h: C(h-1) || B(h);  tail: C3+D
                for t in range(N_SQ):
                    xt = xt0 if t == 0 else load_x(t)
                    emit_B(0, t, xt)
                    for i in range(4):
                        emit_A(xt, t, i)
                for h in range(1, HPG):
                    for t in range(N_SQ):
                        xt = load_x(t)
                        emit_C(h - 1, t)
                        emit_B(h, t, xt)
                    if dbg and h == 1:
                        nc.sync.dma_start(out=dbg_qh[:, :, :], in_=qh_t[:])
                        nc.sync.dma_start(out=dbg_kh[:, :, :], in_=kh_t[:])
                        nc.sync.dma_start(out=dbg_v[:, :, :], in_=v_t[:])
                for sq in range(N_SQ):
                    emit_C(HPG - 1, sq)
                    emit_D(sq)
                if dbg:
                    nc.sync.dma_start(out=dbg_oT[:, :, :], in_=oTb_t[:])

    nc.compile()
    return nc


_PROGRAM = None


def _get_program():
    global _PROGRAM
    if _PROGRAM is None:
        _PROGRAM = build_program()
    return _PROGRAM


def _perm128():
    # even head dims then odd head dims
    return np.concatenate([np.arange(0, HD, 2), np.arange(1, HD, 2)])


def _pack_c(a):
    """[C_in, N] -> [128, NCC, N] with c = cc*128 + p."""
    k, n = a.shape
    return np.ascontiguousarray(
        a.reshape(NCC, 128, n).transpose(1, 0, 2))


def prepare_in_maps(hidden_states, rotary_emb, wq, bq, wk, bk, wv, bv,
                    q_norm_w, k_norm_w, wo, bo):
    f32 = np.float32
    hidden_states = np.asarray(hidden_states, f32)
    rotary_emb = np.asarray(rotary_emb, f32)
    wq, bq = np.asarray(wq, f32), np.asarray(bq, f32)
    wk, bk = np.asarray(wk, f32), np.asarray(bk, f32)
    wv, bv = np.asarray(wv, f32), np.asarray(bv, f32)
    wo = np.asarray(wo, f32)
    q_norm_w, k_norm_w = np.asarray(q_norm_w, f32), np.asarray(k_norm_w, f32)

    p128 = _perm128()
    # rope [2, 2, 64, S] with norm weights folded in
    rope = np.ascontiguousarray(
        rotary_emb[0, :, 0, :, :, :].transpose(2, 3, 1, 0))  # [2, 2, 64, S]
    nwq = q_norm_w[p128].reshape(2, 64)
    nwk = k_norm_w[p128].reshape(2, 64)
    ropeq = (rope * nwq[None, :, :, None]).astype(BF16NP)
    ropek = (rope * nwk[None, :, :, None]).astype(BF16NP)
    onbf = np.ones((1, 128), BF16NP)
    csts = np.zeros((3, 128), f32)
    csts[0, :] = 1.0
    csts[1, :] = EPS
    csts[2, :] = EBIAS

    wqTb = wq.T.astype(BF16NP)
    wkTb = wk.T.astype(BF16NP)
    wvTb = wv.T.astype(BF16NP)
    woTb = wo.T.astype(BF16NP)
    xb = [hidden_states[b].T.astype(BF16NP) for b in range(B)]  # [C, S]

    in_maps = []
    for core in range(N_CORES):
        b, g = divmod(core, HG)
        base = g * GF
        cols = np.concatenate(
            [base + hh * HD + p128 for hh in range(HPG)])
        in_maps.append({
            "xb": _pack_c(xb[b]),
            "wqb": _pack_c(np.ascontiguousarray(wqTb[:, cols])),
            "wkb": _pack_c(np.ascontiguousarray(wkTb[:, cols])),
            "wvb": _pack_c(np.ascontiguousarray(wvTb[:, base:base + GF])),
            "wob": np.ascontiguousarray(
                woTb[base:base + GF, :].reshape(HPG, 128, C)
                .transpose(1, 0, 2)),
            "ropeq": ropeq,
            "ropek": ropek,
            "bq": np.ascontiguousarray(bq[cols]).reshape(HPG, HD),
            "bk": np.ascontiguousarray(bk[cols]).reshape(HPG, HD),
            "onbf": onbf,
            "csts": csts,
        })
    return in_maps


def combine_results(results, bo_eff):
    out = np.zeros((B, S, C), np.float32)
    for core in range(N_CORES):
        b = core // HG
        out[b] += results[core]["out"].astype(np.float32)
    out += bo_eff
    return out


def kernel(hidden_states, rotary_emb, wq, bq, wk, bk, wv, bv,
           q_norm_w, k_norm_w, wo, bo):
    nc = _get_program()
    in_maps = prepare_in_maps(hidden_states, rotary_emb, wq, bq, wk, bk,
                              wv, bv, q_norm_w, k_norm_w, wo, bo)
    res = run_bass_kernel_spmd(nc, in_maps, list(range(N_CORES)))
    # v-bias folded through the output projection: softmax weights sum to
    # one, so attn(v + bv) = attn(v) + bv and out += wo @ bv exactly.
    bo_eff = (np.asarray(bo, np.float64)
              + np.asarray(wo, np.float64) @ np.asarray(bv, np.float64)
              ).astype(np.float32)
    return combine_results(res.results, bo_eff)
